# revision 1
# baseline (speedup 1.0000x reference)
"""Transformer decoder layer (causal self-attn + cross-attn + FFN, 3 post-LNs)
on 8 Trainium2 NeuronCores.

Sharding: 2-way data parallel (batch) x 4-way tensor parallel.
  core c: batch g = c // 4, TP rank r = c % 4.
  - attention: 4 of 16 heads per core (wq/wk/wv column slice 256, wo row
    slice 256), AllReduce[group of 4] after the output projection.
  - FFN: w1 column slice 1024, w2 row slice 1024, AllReduce after w2.
  - residual: each core folds 0.25*residual (+ bias/4) into its partial
    before the AllReduce, so the AllReduce output is directly the LN input.
  - LayerNorms computed redundantly on each core of the group.

On-chip layouts (per core, S tokens):
  feature-major "transposed" activations xT: [128, 8, S] bf16 (E on partitions)
  qT/kT: [128, 2, S] bf16 (head-dim on partitions, 4 heads x 64)
  v:     [128, TB, 4, 65] bf16 token-major, col 64 = ones (rowsum trick)
  attention scores sT: [128 k, 512 q] blocks, softmax along k via exp +
    ones-column rowsums; normalization folded into the o-eviction scale.

Matmul operands are bf16 (f32 PSUM accumulation); residual / LN / collective
payloads are f32.
"""

import numpy as np
import ml_dtypes

import concourse.bass as bass
import concourse.bacc as bacc
import concourse.tile as tile
from concourse import mybir
from concourse import bass_utils
from concourse.masks import make_identity

F32 = mybir.dt.float32
BF16 = mybir.dt.bfloat16
AF = mybir.ActivationFunctionType
ALU = mybir.AluOpType

E = 1024
H_PER_CORE = 4      # heads per core (16 / 4 TP ranks)
DK = 64
QKV = H_PER_CORE * DK   # 256
FFN_SLICE = 1024        # 4096 / 4 TP ranks
EB = E // 128           # 8 E partition-blocks
NEG_BIG = -30000.0      # additive mask value (exp -> 0 in f32)


def _ts(i, n):
    return slice(i * n, (i + 1) * n)


def _pbcast(ap, p=128):
    """Broadcast a 1D DRAM AP across p partitions (partition step 0)."""
    return bass.AP(tensor=ap.tensor, offset=ap.offset, ap=[[0, p]] + list(ap.ap))


def build_decoder_nc(S: int, num_devices: int = 8, stop_after: str | None = None):
    """Build the SPMD Bass program for one core (sequence length S)."""
    assert S % 512 == 0
    TB = S // 128          # token blocks
    QT = S // 512          # query tiles

    nc = bacc.Bacc("TRN2", target_bir_lowering=False, debug=False,
                   num_devices=num_devices)

    # ---------------- DRAM I/O ----------------
    din = {}

    def inp(name, shape, dt):
        din[name] = nc.dram_tensor(name, list(shape), dt, kind="ExternalInput")
        return din[name]

    x0_f = inp("x0_f", [S, E], F32)          # input (this batch), f32
    x0_b = inp("x0_b", [S, E], BF16)         # same, bf16 (for DMA transpose)
    enc_b = inp("enc_b", [S, E], BF16)       # encoder output, bf16

    for p in ("sa", "ca"):
        inp(f"{p}_wq", [E, QKV], BF16)
        inp(f"{p}_wk", [E, QKV], BF16)
        inp(f"{p}_wv", [E, QKV], BF16)
        inp(f"{p}_wo", [QKV, E], BF16)
        inp(f"{p}_bq", [QKV], F32)
        inp(f"{p}_bk", [QKV], F32)
        inp(f"{p}_bv", [QKV], F32)
        inp(f"{p}_bo4", [E], F32)            # bo / group_size
    inp("w1", [E, FFN_SLICE], BF16)
    inp("b1", [FFN_SLICE], F32)
    inp("w2", [FFN_SLICE, E], BF16)
    inp("b24", [E], F32)                     # b2 / group_size
    for i in (1, 2, 3):
        inp(f"ln{i}_g", [E], F32)
        inp(f"ln{i}_b", [E], F32)
    inp("cmask", [4, 128, 512], BF16)        # causal straddle masks

    G = 4 if num_devices >= 8 else num_devices
    out = nc.dram_tensor("out", [S // G, E], F32, kind="ExternalOutput")

    rg = [[0, 1, 2, 3], [4, 5, 6, 7]][: max(1, num_devices // 4)]
    if num_devices < 8:
        rg = [list(range(num_devices))]

    with tile.TileContext(nc) as tc:
        _emit(tc, din, out, S, TB, QT, rg, stop_after)

    nc.compile()
    return nc


PHASES = ["xt", "saqkv", "saattn", "sa", "cakv", "ar1", "ln1",
          "ca", "ar2", "ln2", "ffn1", "ffn2", "full"]


def _emit(tc, din, out, S, TB, QT, rg, stop_after=None):
    nc = tc.nc

    def cut(phase):
        # True -> caller should emit the early-exit and stop
        return stop_after == phase

    with (
        tc.tile_pool(name="const", bufs=1) as const,
        tc.tile_pool(name="wpool", bufs=1) as wpool,
        tc.tile_pool(name="xt", bufs=1) as xt_pool,
        tc.tile_pool(name="qkv", bufs=1) as qkv_pool,
        tc.tile_pool(name="attn", bufs=2) as attn_pool,
        tc.tile_pool(name="opool", bufs=1) as o_pool,
        tc.tile_pool(name="lnp", bufs=2) as lnp,
        tc.tile_pool(name="stat", bufs=8) as stat,
        tc.tile_pool(name="pp", bufs=2, space="PSUM") as pp,
        tc.tile_pool(name="ps_s", bufs=2, space="PSUM") as ps_s,
        tc.tile_pool(name="ps_o", bufs=2, space="PSUM") as ps_o,
        tc.tile_pool(name="ps_t", bufs=2, space="PSUM") as ps_t,
        tc.tile_pool(name="dram", bufs=1, space="DRAM") as dram,
    ):
        # ---------------- constants ----------------
        ident = const.tile([128, 128], BF16)
        make_identity(nc, ident)
        eps_t = const.tile([128, 1], F32)
        nc.vector.memset(eps_t, 1e-12)
        cmask = const.tile([128, 4, 512], BF16)
        nc.sync.dma_start(out=cmask, in_=din["cmask"].ap().rearrange("i p q -> p i q"))

        _bcast_cache = {}

        def bcast(name, dt=F32, tag=""):
            if name in _bcast_cache:
                return _bcast_cache[name]
            t = const.tile([128, E], dt, name=f"bc_{name}", tag=tag)
            nc.sync.dma_start(out=t, in_=_pbcast(din[name].ap()))
            _bcast_cache[name] = t
            return t

        def ln_g(i):
            return bcast(f"ln{i}_g", tag="lng")

        def ln_b(i):
            return bcast(f"ln{i}_b", tag="lnb")

        def bo4(p):
            return bcast(f"{p}_bo4", tag="bo4")

        def b24_b():
            return bcast("b24", tag="bo4")

        # per-partition bias tiles
        def pp_bias(name, nj):
            t = const.tile([128, nj], F32, name=f"ppb_{name}")
            nc.sync.dma_start(out=t, in_=din[name].ap().rearrange("(j p) -> p j", p=128))
            return t

        bq = {p: pp_bias(f"{p}_bq", 2) for p in ("sa", "ca")}
        bk = {p: pp_bias(f"{p}_bk", 2) for p in ("sa", "ca")}
        b1_t = pp_bias("b1", 8)
        def bv_b(p):
            t = const.tile([128, QKV], F32, name=f"bvb_{p}", tag="bvb")
            nc.sync.dma_start(out=t, in_=_pbcast(din[f"{p}_bv"].ap()))
            return t

        # ---------------- DRAM scratch ----------------
        G = len(rg[0])
        ar_in, ar_out = {}, {}
        for i in (1, 2):
            ar_in[i] = dram.tile([S, E], BF16, name=f"ar{i}_in")
            ar_out[i] = dram.tile([S, E], BF16, name=f"ar{i}_out")
        ar_in[3] = dram.tile([S, E], F32, name="ar3_in")
        ar_out[3] = dram.tile([S // G, E], F32, name="rs3_out")
        x_res = {1: dram.tile([S, E], F32, name="x1_dram"),
                 2: dram.tile([S, E], F32, name="x2_dram")}
        x_bf = {1: dram.tile([S, E], BF16, name="x1bf_dram"),
                2: dram.tile([S, E], BF16, name="x2bf_dram")}

        # ---------------- helpers ----------------
        def load_w_qkv(pref):
            w = {}
            for nm in ("wq", "wk", "wv"):
                t = wpool.tile([128, EB, QKV], BF16, tag=nm, name=f"{pref}_{nm}_sb")
                nc.sync.dma_start(out=t, in_=din[f"{pref}_{nm}"].ap().rearrange(
                    "(eb p) m -> p eb m", p=128))
                w[nm] = t
            return w

        def load_w_o(pref):
            t = wpool.tile([128, 2, E], BF16, tag="wo", name=f"{pref}_wo_sb")
            nc.sync.dma_start(out=t, in_=din[f"{pref}_wo"].ap().rearrange(
                "(j p) n -> p j n", p=128))
            return t

        def dma_transpose_in(dst, src_dram):
            # src [S, E] (2-byte) -> dst [128, EB, S] feature-major
            for eb in range(EB):
                nc.sync.dma_start_transpose(dst[:, eb, :], src_dram[:, _ts(eb, 128)])

        def proj_qk(xT, w, b, dst):
            # dst [128, 2, S] bf16 = (w.T @ x.T) + b   (feature-major)
            for j in range(2):
                for tt in range(QT):
                    ps = pp.tile([128, 512], F32, tag="pp")
                    for eb in range(EB):
                        nc.tensor.matmul(ps, w[:, eb, _ts(j, 128)],
                                         xT[:, eb, _ts(tt, 512)],
                                         start=(eb == 0), stop=(eb == EB - 1))
                    nc.vector.tensor_scalar_add(dst[:, j, _ts(tt, 512)], ps,
                                                b[:, j:j + 1])

        def proj_v(xT, w, bvb, dst):
            # dst [128, TB, 4, 65] token-major v (+ ones column)
            nc.vector.memset(dst[:, :, :, 64:65], 1.0)
            for tb in range(TB):
                ps = pp.tile([128, QKV], F32, tag="pp")
                for eb in range(EB):
                    nc.tensor.matmul(ps, xT[:, eb, _ts(tb, 128)], w[:, eb, :],
                                     start=(eb == 0), stop=(eb == EB - 1))
                nc.vector.tensor_add(dst[:, tb, :, 0:64],
                                     ps.rearrange("p (h d) -> p h d", d=64), bvb)

        def attention(qT, kT, v, o_sb, causal):
            for h in range(H_PER_CORE):
                hp = slice((h % 2) * 64, (h % 2) * 64 + 64)
                j = h // 2
                for qt in range(QT):
                    kb_max = min(TB, 4 * qt + 4) if causal else TB
                    at = attn_pool.tile([128, TB, 512], BF16, tag="attn")
                    for kb in range(kb_max):
                        ps = ps_s.tile([128, 512], F32, tag="ps_s")
                        nc.tensor.matmul(ps, kT[hp, j, _ts(kb, 128)],
                                         qT[hp, j, _ts(qt, 512)],
                                         start=True, stop=True)
                        nc.scalar.activation(at[:, kb, :], ps, AF.Exp, scale=0.125)
                        if causal and kb >= 4 * qt:
                            nc.vector.tensor_mul(at[:, kb, :], at[:, kb, :],
                                                 cmask[:, kb - 4 * qt, :])
                    for qs in range(4):
                        po = ps_o.tile([128, 65], F32, tag="ps_o")
                        for kb in range(kb_max):
                            nc.tensor.matmul(po, at[:, kb, _ts(qs, 128)],
                                             v[:, kb, h, :],
                                             start=(kb == 0), stop=(kb == kb_max - 1))
                        rcp = stat.tile([128, 1], F32, tag="rcp")
                        nc.vector.reciprocal(rcp, po[:, 64:65])
                        nc.vector.tensor_scalar_mul(o_sb[:, qt * 4 + qs, h, :],
                                                    po[:, 0:64], rcp)

        def o_transpose(o_sb, oT):
            for tb in range(TB):
                for j in range(2):
                    pt = ps_t.tile([128, 128], BF16, tag="ps_t")
                    nc.tensor.transpose(pt, o_sb[:, tb, 2 * j:2 * j + 2, :], ident)
                    nc.vector.tensor_copy(oT[:, j, _ts(tb, 128)], pt)

        def out_proj(oT, wo, bo4_b, ar_dst):
            # bf16 partial = oT.T @ wo + bo/G -> ar_dst (residual added post-AR)
            for tb in range(TB):
                y = lnp.tile([128, E], BF16, tag="res_out")
                for nh in range(2):
                    ps = pp.tile([128, 512], F32, tag="pp")
                    for j in range(2):
                        nc.tensor.matmul(ps, oT[:, j, _ts(tb, 128)],
                                         wo[:, j, _ts(nh, 512)],
                                         start=(j == 0), stop=(j == 1))
                    nc.vector.tensor_add(y[:, _ts(nh, 512)], ps,
                                         bo4_b[:, _ts(nh, 512)])
                nc.sync.dma_start(out=ar_dst[_ts(tb, 128), :], in_=y)

        def all_reduce(i):
            nc.gpsimd.collective_compute(
                "AllReduce", ALU.add, replica_groups=rg,
                ins=[ar_in[i].opt()], outs=[ar_out[i].opt()])

        def reduce_scatter(i):
            nc.gpsimd.collective_compute(
                "ReduceScatter", ALU.add, replica_groups=rg,
                ins=[ar_in[i].opt()], outs=[ar_out[i].opt()])

        def layer_norm(i, make_bf, to_out=None, residual_src=None, n_blocks=None):
            # LN over (ar_out[i] [+ residual]); writes x_res[i]/x_bf[i] or `out`
            for tb in range(n_blocks if n_blocks is not None else TB):
                ld = lnp.tile([128, E], F32, tag="ln_io")
                if residual_src is not None:
                    arb = lnp.tile([128, E], BF16, tag="ln_bf")
                    nc.sync.dma_start(out=arb, in_=ar_out[i][_ts(tb, 128), :])
                    nc.sync.dma_start(out=ld, in_=residual_src[_ts(tb, 128), :])
                    nc.vector.tensor_add(ld, ld, arb)
                else:
                    nc.sync.dma_start(out=ld, in_=ar_out[i][_ts(tb, 128), :])
                st = stat.tile([128, 2, 6], F32, tag="bnst")
                for sg in range(2):
                    nc.vector.bn_stats(st[:, sg, :], ld[:, _ts(sg, 512)])
                mv = stat.tile([128, 2], F32, tag="bnmv")
                nc.vector.bn_aggr(mv, st)
                sd = stat.tile([128, 1], F32, tag="sd")
                nc.scalar.activation(sd, mv[:, 1:2], AF.Sqrt, bias=eps_t)
                rstd = stat.tile([128, 1], F32, tag="rstd")
                nc.vector.reciprocal(rstd, sd)
                nc.vector.tensor_scalar(ld, ld, mv[:, 0:1], rstd,
                                        ALU.subtract, ALU.mult)
                nc.vector.tensor_mul(ld, ld, ln_g(i))
                nc.vector.tensor_add(ld, ld, ln_b(i))
                xf = ld
                if to_out is not None:
                    nc.sync.dma_start(out=to_out[_ts(tb, 128), :], in_=xf)
                else:
                    nc.sync.dma_start(out=x_res[i][_ts(tb, 128), :], in_=xf)
                    if make_bf:
                        xb = lnp.tile([128, E], BF16, tag="ln_bf")
                        nc.vector.tensor_copy(xb, xf)
                        nc.sync.dma_start(out=x_bf[i][_ts(tb, 128), :], in_=xb)

        # ================= self-attention =================
        def finish():
            nc.sync.dma_start(out=out.ap(), in_=din["x0_f"].ap()[:S // len(rg[0]), :])

        if cut("null"):
            finish()
            return

        x0T = xt_pool.tile([128, EB, S], BF16, tag="xT", name="x0T")
        dma_transpose_in(x0T, din["x0_b"].ap())

        sa_w = load_w_qkv("sa")
        sa_wo = load_w_o("sa")

        qT = qkv_pool.tile([128, 2, S], BF16, tag="qT", name="sa_qT")
        kT = qkv_pool.tile([128, 2, S], BF16, tag="kT", name="sa_kT")
        v = qkv_pool.tile([128, TB, 4, 65], BF16, tag="v", name="sa_v")
        proj_qk(x0T, sa_w["wq"], bq["sa"], qT)
        proj_qk(x0T, sa_w["wk"], bk["sa"], kT)
        proj_v(x0T, sa_w["wv"], bv_b("sa"), v)

        if cut("saqkv"):
            finish()
            return

        # encoder transpose-load takes over x0T's slot once SA projections drain
        encT = xt_pool.tile([128, EB, S], BF16, tag="xT", name="encT")
        dma_transpose_in(encT, din["enc_b"].ap())

        if cut("xt"):
            finish()
            return

        o_sb = o_pool.tile([128, TB, 4, 64], BF16, tag="o", name="sa_o")
        attention(qT, kT, v, o_sb, causal=True)
        oT = qkv_pool.tile([128, 2, S], BF16, tag="qT", name="sa_oT")
        o_transpose(o_sb, oT)

        if cut("saattn"):
            finish()
            return
        out_proj(oT, sa_wo, bo4("sa"), ar_in[1])

        if cut("sa"):
            finish()
            return

        # cross-attention K/V from encoder (independent of AR1 -> overlaps it)
        ca_w = load_w_qkv("ca")
        ca_kT = qkv_pool.tile([128, 2, S], BF16, tag="kT", name="ca_kT")
        ca_v = qkv_pool.tile([128, TB, 4, 65], BF16, tag="v", name="ca_v")
        proj_qk(encT, ca_w["wk"], bk["ca"], ca_kT)
        proj_v(encT, ca_w["wv"], bv_b("ca"), ca_v)

        if cut("cakv"):
            finish()
            return

        all_reduce(1)

        if cut("ar1"):
            finish()
            return
        layer_norm(1, make_bf=True, residual_src=din["x0_f"].ap())

        # ================= cross-attention =================
        x1T = xt_pool.tile([128, EB, S], BF16, tag="xT", name="x1T")
        dma_transpose_in(x1T, x_bf[1])

        if cut("ln1"):
            finish()
            return
        ca_wo = load_w_o("ca")
        ca_qT = qkv_pool.tile([128, 2, S], BF16, tag="qT", name="ca_qT")
        proj_qk(x1T, ca_w["wq"], bq["ca"], ca_qT)

        ca_o = o_pool.tile([128, TB, 4, 64], BF16, tag="o", name="ca_o")
        attention(ca_qT, ca_kT, ca_v, ca_o, causal=False)
        ca_oT = qkv_pool.tile([128, 2, S], BF16, tag="qT", name="ca_oT")
        o_transpose(ca_o, ca_oT)
        out_proj(ca_oT, ca_wo, bo4("ca"), ar_in[2])

        if cut("ca"):
            finish()
            return

        # FFN weights load early (overlaps AR2)
        w1_sb = wpool.tile([128, EB, FFN_SLICE], BF16, tag="wk")
        nc.sync.dma_start(out=w1_sb, in_=din["w1"].ap().rearrange(
            "(eb p) m -> p eb m", p=128))
        w2_sb = wpool.tile([128, 8, E], BF16, tag="wq")
        nc.sync.dma_start(out=w2_sb, in_=din["w2"].ap().rearrange(
            "(fb p) n -> p fb n", p=128))

        all_reduce(2)

        if cut("ar2"):
            finish()
            return
        layer_norm(2, make_bf=True, residual_src=x_res[1])

        # ================= FFN =================
        x2T = xt_pool.tile([128, EB, S], BF16, tag="xT", name="x2T")
        dma_transpose_in(x2T, x_bf[2])

        if cut("ln2"):
            finish()
            return
        hT = xt_pool.tile([128, 8, S], BF16, tag="hT", name="hT")
        for fb in range(8):
            for tt in range(QT):
                ps = pp.tile([128, 512], F32, tag="pp")
                for eb in range(EB):
                    nc.tensor.matmul(ps, w1_sb[:, eb, _ts(fb, 128)],
                                     x2T[:, eb, _ts(tt, 512)],
                                     start=(eb == 0), stop=(eb == EB - 1))
                nc.scalar.activation(hT[:, fb, _ts(tt, 512)], ps, AF.Relu,
                                     bias=b1_t[:, fb:fb + 1])

        if cut("ffn1"):
            finish()
            return
        for tb in range(TB):
            res = lnp.tile([128, E], F32, tag="ln_io")
            nc.sync.dma_start(out=res, in_=x_res[2][_ts(tb, 128), :])
            nc.vector.scalar_tensor_tensor(res, res, 1.0 / len(rg[0]),
                                           b24_b(), ALU.mult, ALU.add)
            base = res
            y = lnp.tile([128, E], F32, tag="res_out")
            for nh in range(2):
                ps = pp.tile([128, 512], F32, tag="pp")
                for fb in range(8):
                    nc.tensor.matmul(ps, hT[:, fb, _ts(tb, 128)],
                                     w2_sb[:, fb, _ts(nh, 512)],
                                     start=(fb == 0), stop=(fb == 7))
                nc.vector.tensor_add(y[:, _ts(nh, 512)], base[:, _ts(nh, 512)], ps)
            nc.sync.dma_start(out=ar_in[3][_ts(tb, 128), :], in_=y)

        if cut("ffn2"):
            finish()
            return

        reduce_scatter(3)
        layer_norm(3, make_bf=False, to_out=out.ap(), n_blocks=TB // G)


# ====================== host side ======================

def make_causal_masks():
    # mask_i[pk, pq] = 1.0 if pk <= pq - 128*i else 0  (straddle blocks)
    m = np.zeros((4, 128, 512), dtype=np.float32)
    pk = np.arange(128)[:, None]
    pq = np.arange(512)[None, :]
    for i in range(4):
        m[i] = (pk <= pq - 128 * i).astype(np.float32)
    return m.astype(ml_dtypes.bfloat16)


def shard_inputs(inputs, num_devices=8):
    """Full inputs (reference.setup_inputs keys) -> per-core in_maps."""
    bf = ml_dtypes.bfloat16
    f32 = np.float32
    G = 4 if num_devices >= 8 else num_devices
    cmask = make_causal_masks()
    in_maps = []
    inp = {k: np.asarray(v) for k, v in inputs.items()}
    for c in range(num_devices):
        g = c // G if num_devices >= 8 else 0
        r = c % G
        qs = slice(r * QKV, (r + 1) * QKV)
        fs = slice(r * FFN_SLICE, (r + 1) * FFN_SLICE)
        x0 = inp["input"][g].astype(f32)
        m = {
            "x0_f": x0,
            "x0_b": x0.astype(bf),
            "enc_b": inp["encoder_output"][g].astype(bf),
            "w1": inp["ffn_w1"][:, fs].astype(bf),
            "b1": inp["ffn_b1"][fs].astype(f32),
            "w2": inp["ffn_w2"][fs, :].astype(bf),
            "b24": (inp["ffn_b2"] / G).astype(f32),
            "cmask": cmask,
        }
        for p in ("sa", "ca"):
            m[f"{p}_wq"] = inp[f"{p}_wq"][:, qs].astype(bf)
            m[f"{p}_wk"] = inp[f"{p}_wk"][:, qs].astype(bf)
            m[f"{p}_wv"] = inp[f"{p}_wv"][:, qs].astype(bf)
            m[f"{p}_wo"] = inp[f"{p}_wo"][qs, :].astype(bf)
            m[f"{p}_bq"] = inp[f"{p}_bq"][qs].astype(f32)
            m[f"{p}_bk"] = inp[f"{p}_bk"][qs].astype(f32)
            m[f"{p}_bv"] = inp[f"{p}_bv"][qs].astype(f32)
            m[f"{p}_bo4"] = (inp[f"{p}_bo"] / G).astype(f32)
        for i in (1, 2, 3):
            m[f"ln{i}_g"] = inp[f"ln{i}_g"].astype(f32)
            m[f"ln{i}_b"] = inp[f"ln{i}_b"].astype(f32)
        in_maps.append(m)
    return in_maps


_NC_CACHE = {}


def _get_nc(S):
    if S not in _NC_CACHE:
        _NC_CACHE[S] = build_decoder_nc(S)
    return _NC_CACHE[S]


def kernel(**inputs):
    x = np.asarray(inputs["input"])
    B, S, _ = x.shape
    nc = _get_nc(S)
    in_maps = shard_inputs(inputs)
    res = bass_utils.run_bass_kernel_spmd(nc, in_maps, core_ids=list(range(8)))
    outb = [np.concatenate([res.results[g * 4 + r]["out"] for r in range(4)], axis=0)
            for g in range(B)]
    return np.stack(outb, axis=0).astype(np.float32)



# revision 13
# speedup vs baseline: 6.9300x; 6.9300x over previous
"""Transformer decoder layer (causal self-attn + cross-attn + FFN, 3 post-LNs)
on 8 Trainium2 NeuronCores — single-collective design.

Sharding: 2-way data parallel (batch) x 4-way within each batch group.
  core c: batch g = c // 4, rank r = c % 4; chunk = tokens [r*512, (r+1)*512).
  - self-attention: tensor parallel over heads (4 of 16 per core),
    ReduceScatter after the output projection -> each core holds its
    512-token chunk of the attention output (the ONLY collective).
  - LN1/LN2/LN3: chunk-local.
  - cross-attention: sequence parallel — all 16 heads for the own 512-token
    query chunk; encoder K/V computed full-width on every core (overlaps
    the ReduceScatter window). No collective needed.
  - FFN: chunk-local with full weights (w1/w2 streamed/parked in slots
    vacated by earlier phases). No collective needed.
  - output: each core writes its own [512, E] chunk; host concatenates.

Host passes x0 and enc pre-transposed ([E, S] feature-major) so no DMA
transposes are needed; x1/x2 chunk transposes are done on the PE.

Attention: scores [128 k, 512 q] blocks, exp via scalar engine, softmax
along k with a ones-column rowsum folded into the o-eviction scale;
probability tiles are streamed (4-deep pool) into the PV accumulation.
"""

import numpy as np
import ml_dtypes

import concourse.bass as bass
import concourse.bacc as bacc
import concourse.tile as tile
from concourse import mybir
from concourse import bass_utils
from concourse.masks import make_identity

F32 = mybir.dt.float32
BF16 = mybir.dt.bfloat16
AF = mybir.ActivationFunctionType
ALU = mybir.AluOpType

E = 1024
H = 16                  # total heads
SA_HPC = 4              # SA heads per core (16 / 4 ranks)
DK = 64
SA_QKV = SA_HPC * DK    # 256
EB = E // 128           # 8 E partition-blocks
FH = 4096               # FFN hidden


def _ts(i, n):
    return slice(i * n, (i + 1) * n)


def _pbcast(ap, p=128):
    """Broadcast a 1D DRAM AP across p partitions (partition step 0)."""
    return bass.AP(tensor=ap.tensor, offset=ap.offset, ap=[[0, p]] + list(ap.ap))


PHASES = ["null", "saqkv", "sa", "cakv", "rs1", "ln1", "caq", "ca",
          "ln2", "ffn1", "full"]


def build_decoder_nc(S: int, num_devices: int = 8, stop_after: str | None = None,
                     repeat: int = 1):
    assert S % 512 == 0
    nc = bacc.Bacc("TRN2", target_bir_lowering=False, debug=False,
                   num_devices=num_devices)

    G = 4
    CH = S // G            # chunk tokens per core

    din = {}

    def inp(name, shape, dt):
        din[name] = nc.dram_tensor(name, list(shape), dt, kind="ExternalInput")
        return din[name]

    inp("x0T_b", [E, S], BF16)
    inp("x0c_f", [CH, E], F32)
    inp("encT_b", [E, S], BF16)

    inp("sa_wq", [E, SA_QKV], BF16)
    inp("sa_wk", [E, SA_QKV], BF16)
    inp("sa_wv", [E, SA_QKV], BF16)
    inp("sa_wo", [SA_QKV, E], BF16)
    inp("sa_bq", [SA_QKV], F32)
    inp("sa_bk", [SA_QKV], F32)
    inp("sa_bv", [SA_QKV], F32)
    inp("sa_bo4", [E], F32)          # bo / 4 (summed by the ReduceScatter)

    inp("ca_wq", [E, E], BF16)
    inp("ca_wk", [E, E], BF16)
    inp("ca_wv", [E, E], BF16)
    inp("ca_wo", [E, E], BF16)
    inp("ca_bq", [E], F32)
    inp("ca_bk", [E], F32)
    inp("ca_bv", [E], F32)
    inp("ca_bo", [E], F32)

    inp("w1", [E, FH], BF16)
    inp("b1", [FH], F32)
    inp("w2", [FH, E], BF16)
    inp("b2", [E], F32)
    for i in (1, 2, 3):
        inp(f"ln{i}_g", [E], F32)
        inp(f"ln{i}_b", [E], F32)
    inp("cmask", [4, 128, 512], BF16)

    out = nc.dram_tensor("out", [CH, E], F32, kind="ExternalOutput")

    rg = [[0, 1, 2, 3], [4, 5, 6, 7]][: max(1, num_devices // 4)]
    if num_devices < 8:
        rg = [list(range(num_devices))]

    with tile.TileContext(nc) as tc:
        for rep in range(repeat):
            _emit(tc, din, out, S, rg, stop_after, sfx=f"_r{rep}")

    nc.compile()
    return nc


def _emit(tc, din, out, S, rg, stop_after=None, sfx=""):
    nc = tc.nc
    TB = S // 128          # k token blocks
    QT = S // 512          # SA query tiles
    CH = S // 4            # chunk tokens
    CB = CH // 128         # chunk token blocks

    def cut(phase):
        return stop_after == phase

    def finish():
        nc.sync.dma_start(out=out.ap(), in_=din["x0c_f"].ap())

    with (
        tc.tile_pool(name="const" + sfx, bufs=1) as const,
        tc.tile_pool(name="wpool" + sfx, bufs=1) as wpool,
        tc.tile_pool(name="bigA" + sfx, bufs=1) as bigA,
        tc.tile_pool(name="bigB" + sfx, bufs=1) as bigB,
        tc.tile_pool(name="bigC" + sfx, bufs=1) as bigC,
        tc.tile_pool(name="qk" + sfx, bufs=1) as qk_pool,
        tc.tile_pool(name="at" + sfx, bufs=4) as at_pool,
        tc.tile_pool(name="opool" + sfx, bufs=1) as o_pool,
        tc.tile_pool(name="ws" + sfx, bufs=2) as ws,
        tc.tile_pool(name="xtc" + sfx, bufs=1) as xtc_pool,
        tc.tile_pool(name="lnp" + sfx, bufs=2) as lnp,
        tc.tile_pool(name="stat" + sfx, bufs=8) as stat,
        tc.tile_pool(name="pp" + sfx, bufs=2, space="PSUM") as pp,
        tc.tile_pool(name="ps_s" + sfx, bufs=2, space="PSUM") as ps_s,
        tc.tile_pool(name="po" + sfx, bufs=2, space="PSUM") as po_pool,
        tc.tile_pool(name="dram" + sfx, bufs=1, space="DRAM") as dram,
    ):
        # ---------------- constants ----------------
        ident = const.tile([128, 128], BF16)
        make_identity(nc, ident)
        eps_t = const.tile([128, 1], F32)
        nc.vector.memset(eps_t, 1e-12)
        cmask = const.tile([128, 4, 512], BF16)
        nc.sync.dma_start(out=cmask, in_=din["cmask"].ap().rearrange("i p q -> p i q"))

        _bcast_cache = {}

        def bcast(name, tag):
            if name in _bcast_cache:
                return _bcast_cache[name]
            t = const.tile([128, E], F32, name=f"bc_{name}{sfx}", tag=tag)
            nc.sync.dma_start(out=t, in_=_pbcast(din[name].ap()))
            _bcast_cache[name] = t
            return t

        def pp_bias(name, nj):
            t = const.tile([128, nj], F32, name=f"ppb_{name}{sfx}", tag=f"ppb_{name}")
            nc.sync.dma_start(out=t, in_=din[name].ap().rearrange("(j p) -> p j", p=128))
            return t

        sa_bq = pp_bias("sa_bq", 2)
        sa_bk = pp_bias("sa_bk", 2)
        ca_bq = pp_bias("ca_bq", 8)
        ca_bk = pp_bias("ca_bk", 8)
        b1_t = pp_bias("b1", FH // 128)

        def bvb(name, n):
            t = const.tile([128, n], F32, name=f"bvb_{name}{sfx}", tag="bvb")
            nc.sync.dma_start(out=t, in_=_pbcast(din[name].ap()))
            return t

        # ---------------- DRAM scratch ----------------
        ar1_in = dram.tile([S, E], BF16, name="ar1_in" + sfx)
        rs1_out = dram.tile([CH, E], BF16, name="rs1_out" + sfx)
        x1c_d = dram.tile([CH, E], F32, name="x1c" + sfx)
        x2c_d = dram.tile([CH, E], F32, name="x2c" + sfx)

        # ---------------- helpers ----------------
        def proj_qk(xT, w, b, dst, nj, nt):
            # dst [128, nj, nt*512] feature-major = w.T @ xT + b
            for j in range(nj):
                for tt in range(nt):
                    ps = pp.tile([128, 512], F32, tag="pp")
                    for eb in range(EB):
                        nc.tensor.matmul(ps, w[:, eb, _ts(j, 128)],
                                         xT[:, eb, _ts(tt, 512)],
                                         start=(eb == 0), stop=(eb == EB - 1))
                    nc.scalar.activation(dst[:, j, _ts(tt, 512)], ps, AF.Identity,
                                         bias=b[:, j:j + 1])

        def proj_v(xT, w_dram, bvb_t, dst, nh):
            # dst [128, TB, nh, 65] token-major (+ ones col); w streamed vw-wide
            nc.vector.memset(dst[:, :, :, 64:65], 1.0)
            vw = min(512, nh * DK)
            for vh in range((nh * DK) // vw):
                wv = ws.tile([128, EB, vw], BF16, tag="ws")
                nc.sync.dma_start(out=wv, in_=w_dram[:, _ts(vh, vw)].rearrange(
                    "(eb p) m -> p eb m", p=128))
                for tb in range(TB):
                    ps = pp.tile([128, vw], F32, tag="pp")
                    for eb in range(EB):
                        nc.tensor.matmul(ps, xT[:, eb, _ts(tb, 128)], wv[:, eb, :],
                                         start=(eb == 0), stop=(eb == EB - 1))
                    hs = slice(vh * (vw // DK), (vh + 1) * (vw // DK))
                    nc.vector.tensor_add(dst[:, tb, hs, 0:64],
                                         ps.rearrange("p (h d) -> p h d", d=64),
                                         bvb_t[:, _ts(vh, vw)].rearrange(
                                             "p (h d) -> p h d", d=64))

        def attention(qT, kT, v, o_sb, nh, nqt, causal):
            # qT [128, nh/2, nqt*512]; kT [128, nh/2, S]; v [128, TB, nh, 65]
            # o_sb [128, nqt*4, nh, 64].  kb processed in pairs: scores land in
            # a 2-bank PSUM tile and one [128, 1024] exp covers both blocks.
            for h in range(nh):
                hp = slice((h % 2) * 64, (h % 2) * 64 + 64)
                j = h // 2
                for qt in range(nqt):
                    kb_max = min(TB, 4 * qt + 4) if causal else TB
                    po = po_pool.tile([128, 4, 128], F32, tag="po")
                    for kp in range(kb_max // 2):
                        ps2 = ps_s.tile([128, 2, 512], F32, tag="ps_s")
                        for z in range(2):
                            nc.tensor.matmul(ps2[:, z, :],
                                             kT[hp, j, _ts(2 * kp + z, 128)],
                                             qT[hp, j, _ts(qt, 512)],
                                             start=True, stop=True)
                        at = at_pool.tile([128, 2, 512], BF16, tag="at")
                        nc.scalar.activation(at, ps2, AF.Exp, scale=0.125)
                        for z in range(2):
                            kb = 2 * kp + z
                            if causal and kb >= 4 * qt:
                                nc.vector.tensor_mul(at[:, z, :], at[:, z, :],
                                                     cmask[:, kb - 4 * qt, :])
                            for qs in range(4):
                                # one accumulation group for the whole po tile:
                                # start clears has_written for the bank; each
                                # element's first write overwrites, rest add.
                                nc.tensor.matmul(po[:, qs, 0:65],
                                                 at[:, z, _ts(qs, 128)],
                                                 v[:, kb, h, :],
                                                 start=(kb == 0 and qs == 0),
                                                 stop=(kb == kb_max - 1 and qs == 3))
                    for qs in range(4):
                        rcp = stat.tile([128, 1], F32, tag="rcp")
                        nc.vector.reciprocal(rcp, po[:, qs, 64:65])
                        nc.vector.tensor_scalar_mul(o_sb[:, qt * 4 + qs, h, :],
                                                    po[:, qs, 0:64], rcp)

        def o_transpose(o_sb, oT, ntb):
            # o_sb [128, ntb, nh, 64] -> oT [128, nh*64/128, ntb*128]
            nh = o_sb.shape[2]
            for tb in range(ntb):
                for jj in range(nh // 2):
                    pt = po_pool.tile([128, 128], BF16, tag="po")
                    nc.tensor.transpose(pt, o_sb[:, tb, 2 * jj:2 * jj + 2, :], ident)
                    nc.vector.tensor_copy(oT[:, jj, _ts(tb, 128)], pt)

        def ln_tile(ld, i):
            # in-place layernorm of ld [128, E] with ln{i} params
            st = stat.tile([128, 2, 6], F32, tag="bnst")
            for sg in range(2):
                nc.vector.bn_stats(st[:, sg, :], ld[:, _ts(sg, 512)])
            mv = stat.tile([128, 2], F32, tag="bnmv")
            nc.vector.bn_aggr(mv, st)
            sd = stat.tile([128, 1], F32, tag="sd")
            nc.scalar.activation(sd, mv[:, 1:2], AF.Sqrt, bias=eps_t)
            rstd = stat.tile([128, 1], F32, tag="rstd")
            nc.vector.reciprocal(rstd, sd)
            nc.vector.tensor_scalar(ld, ld, mv[:, 0:1], rstd,
                                    ALU.subtract, ALU.mult)
            nc.vector.tensor_mul(ld, ld, bcast(f"ln{i}_g", "lng"))
            nc.vector.tensor_add(ld, ld, bcast(f"ln{i}_b", "lnb"))

        def transpose_chunk(xb_tb, xTc, tb):
            # xb_tb [128, E] bf16 token-major -> xTc[:, eb, tb*128:...]
            for eb in range(EB):
                pt = po_pool.tile([128, 128], BF16, tag="po")
                nc.tensor.transpose(pt, xb_tb[:, _ts(eb, 128)], ident)
                nc.vector.tensor_copy(xTc[:, eb, _ts(tb, 128)], pt)

        # ================= SA (TP over heads) =================
        if cut("null"):
            finish()
            return

        x0T = bigA.tile([128, EB, S], BF16, tag="bigA", name="x0T" + sfx)
        nc.sync.dma_start(out=x0T, in_=din["x0T_b"].ap().rearrange(
            "(eb p) s -> p eb s", p=128))
        encT = bigB.tile([128, EB, S], BF16, tag="bigB", name="encT" + sfx)
        nc.sync.dma_start(out=encT, in_=din["encT_b"].ap().rearrange(
            "(eb p) s -> p eb s", p=128))

        sa_w = {}
        for nm in ("wq", "wk", "wv"):
            t = wpool.tile([128, EB, SA_QKV], BF16, tag=nm, name=f"sa_{nm}{sfx}")
            nc.sync.dma_start(out=t, in_=din[f"sa_{nm}"].ap().rearrange(
                "(eb p) m -> p eb m", p=128))
            sa_w[nm] = t
        sa_wo = wpool.tile([128, 2, E], BF16, tag="wo", name=f"sa_wo{sfx}")
        nc.sync.dma_start(out=sa_wo, in_=din["sa_wo"].ap().rearrange(
            "(j p) n -> p j n", p=128))

        qT = qk_pool.tile([128, 2, S], BF16, tag="qT", name="sa_qT" + sfx)
        kT = qk_pool.tile([128, 2, S], BF16, tag="kT", name="sa_kT" + sfx)
        v = bigC.tile([128, TB, SA_HPC, 65], BF16, tag="bigC", name="sa_v" + sfx)
        proj_qk(x0T, sa_w["wq"], sa_bq, qT, 2, QT)
        proj_qk(x0T, sa_w["wk"], sa_bk, kT, 2, QT)
        proj_v(x0T, din["sa_wv"].ap(), bvb("sa_bv", SA_QKV), v, SA_HPC)

        if cut("saqkv"):
            finish()
            return

        o_sb = o_pool.tile([128, TB, SA_HPC, 64], BF16, tag="o", name="sa_o" + sfx)
        attention(qT, kT, v, o_sb, SA_HPC, QT, causal=True)
        oT = qk_pool.tile([128, 2, S], BF16, tag="qT", name="sa_oT" + sfx)
        o_transpose(o_sb, oT, TB)

        # out_proj partials (+ bo/4) -> ar1_in
        bo4 = bcast("sa_bo4", "bo")
        for tb in range(TB):
            y = lnp.tile([128, E], BF16, tag="res_out")
            for nh2 in range(2):
                ps = pp.tile([128, 512], F32, tag="pp")
                for jj in range(2):
                    nc.tensor.matmul(ps, oT[:, jj, _ts(tb, 128)],
                                     sa_wo[:, jj, _ts(nh2, 512)],
                                     start=(jj == 0), stop=(jj == 1))
                nc.vector.tensor_add(y[:, _ts(nh2, 512)], ps, bo4[:, _ts(nh2, 512)])
            nc.sync.dma_start(out=ar1_in[_ts(tb, 128), :], in_=y)

        if cut("sa"):
            finish()
            return

        # ========== encoder K/V (full width; overlaps RS1) ==========
        ekT = bigA.tile([128, EB, S], BF16, tag="bigA", name="ekT" + sfx)
        for j in range(EB):
            wk = ws.tile([128, EB, 128], BF16, tag="ws")
            nc.sync.dma_start(out=wk, in_=din["ca_wk"].ap()[:, _ts(j, 128)].rearrange(
                "(eb p) m -> p eb m", p=128))
            for tt in range(QT):
                ps = pp.tile([128, 512], F32, tag="pp")
                for eb in range(EB):
                    nc.tensor.matmul(ps, wk[:, eb, :], encT[:, eb, _ts(tt, 512)],
                                     start=(eb == 0), stop=(eb == EB - 1))
                nc.scalar.activation(ekT[:, j, _ts(tt, 512)], ps, AF.Identity,
                                     bias=ca_bk[:, j:j + 1])
        ev = bigC.tile([128, TB, H, 65], BF16, tag="bigC", name="ev" + sfx)
        proj_v(encT, din["ca_wv"].ap(), bvb("ca_bv", E), ev, H)

        if cut("cakv"):
            finish()
            return

        nc.gpsimd.collective_compute(
            "ReduceScatter", ALU.add, replica_groups=rg,
            ins=[ar1_in.opt()], outs=[rs1_out.opt()])

        if cut("rs1"):
            finish()
            return

        # ========== LN1 (chunk) + x1 transpose ==========
        x1Tc = xtc_pool.tile([128, EB, CH], BF16, tag="xtc", name="x1Tc" + sfx)
        for tb in range(CB):
            ld = lnp.tile([128, E], F32, tag="ln_io")
            arb = lnp.tile([128, E], BF16, tag="ln_bf")
            nc.sync.dma_start(out=arb, in_=rs1_out[_ts(tb, 128), :])
            nc.sync.dma_start(out=ld, in_=din["x0c_f"].ap()[_ts(tb, 128), :])
            nc.vector.tensor_add(ld, ld, arb)
            ln_tile(ld, 1)
            nc.sync.dma_start(out=x1c_d[_ts(tb, 128), :], in_=ld)
            xb = lnp.tile([128, E], BF16, tag="ln_bf")
            nc.vector.tensor_copy(xb, ld)
            transpose_chunk(xb, x1Tc, tb)

        if cut("ln1"):
            finish()
            return

        # ========== CA (sequence parallel, all heads) ==========
        qTc = qk_pool.tile([128, EB, CH], BF16, tag="qT", name="ca_qTc" + sfx)
        for j in range(EB):
            wq = ws.tile([128, EB, 128], BF16, tag="ws")
            nc.sync.dma_start(out=wq, in_=din["ca_wq"].ap()[:, _ts(j, 128)].rearrange(
                "(eb p) m -> p eb m", p=128))
            ps = pp.tile([128, CH], F32, tag="pp")
            for eb in range(EB):
                nc.tensor.matmul(ps, wq[:, eb, :], x1Tc[:, eb, :],
                                 start=(eb == 0), stop=(eb == EB - 1))
            nc.scalar.activation(qTc[:, j, :], ps, AF.Identity,
                                 bias=ca_bq[:, j:j + 1])

        if cut("caq"):
            finish()
            return

        o_c = o_pool.tile([128, CB, H, 64], BF16, tag="o", name="ca_oc" + sfx)
        attention(qTc, ekT, ev, o_c, H, 1, causal=False)
        oTc = qk_pool.tile([128, EB, CH], BF16, tag="qT", name="ca_oTc" + sfx)
        o_transpose(o_c, oTc, CB)

        if cut("ca"):
            finish()
            return

        # ========== CA out_proj + LN2 (chunk) + x2 transpose ==========
        wo0 = ws.tile([128, EB, 512], BF16, tag="ws")
        nc.sync.dma_start(out=wo0, in_=din["ca_wo"].ap()[:, 0:512].rearrange(
            "(j p) n -> p j n", p=128))
        wo1 = ws.tile([128, EB, 512], BF16, tag="ws")
        nc.sync.dma_start(out=wo1, in_=din["ca_wo"].ap()[:, 512:1024].rearrange(
            "(j p) n -> p j n", p=128))
        ca_bo = bcast("ca_bo", "bo")
        x2Tc = xtc_pool.tile([128, EB, CH], BF16, tag="xtc", name="x2Tc" + sfx)
        for tb in range(CB):
            ld = lnp.tile([128, E], F32, tag="ln_io")
            nc.sync.dma_start(out=ld, in_=x1c_d[_ts(tb, 128), :])
            nc.vector.tensor_add(ld, ld, ca_bo)
            for nh2, wo in ((0, wo0), (1, wo1)):
                ps = pp.tile([128, 512], F32, tag="pp")
                for jj in range(EB):
                    nc.tensor.matmul(ps, oTc[:, jj, _ts(tb, 128)], wo[:, jj, :],
                                     start=(jj == 0), stop=(jj == EB - 1))
                nc.vector.tensor_add(ld[:, _ts(nh2, 512)], ld[:, _ts(nh2, 512)], ps)
            ln_tile(ld, 2)
            nc.sync.dma_start(out=x2c_d[_ts(tb, 128), :], in_=ld)
            xb = lnp.tile([128, E], BF16, tag="ln_bf")
            nc.vector.tensor_copy(xb, ld)
            transpose_chunk(xb, x2Tc, tb)

        if cut("ln2"):
            finish()
            return

        # ========== FFN (chunk-local, full weights) ==========
        # w2 parks in the slots vacated by ekT (bigA) and encT (bigB)
        w2a = bigA.tile([128, 16, E], BF16, tag="bigA", name="w2a" + sfx)
        nc.sync.dma_start(out=w2a, in_=din["w2"].ap()[0:2048, :].rearrange(
            "(hb p) n -> p hb n", p=128))
        w2b = bigB.tile([128, 16, E], BF16, tag="bigB", name="w2b" + sfx)
        nc.sync.dma_start(out=w2b, in_=din["w2"].ap()[2048:4096, :].rearrange(
            "(hb p) n -> p hb n", p=128))

        hT = bigC.tile([128, FH // 128, CH], BF16, tag="bigC", name="hT" + sfx)
        for hc in range(FH // 512):
            w1c = ws.tile([128, EB, 512], BF16, tag="ws")
            nc.sync.dma_start(out=w1c, in_=din["w1"].ap()[:, _ts(hc, 512)].rearrange(
                "(eb p) m -> p eb m", p=128))
            for hl in range(4):
                hb = hc * 4 + hl
                ps = pp.tile([128, 512], F32, tag="pp")
                for eb in range(EB):
                    nc.tensor.matmul(ps, w1c[:, eb, _ts(hl, 128)], x2Tc[:, eb, :],
                                     start=(eb == 0), stop=(eb == EB - 1))
                nc.scalar.activation(hT[:, hb, :], ps, AF.Relu,
                                     bias=b1_t[:, hb:hb + 1])

        if cut("ffn1"):
            finish()
            return

        b2 = bcast("b2", "bo")
        for tb in range(CB):
            ld = lnp.tile([128, E], F32, tag="ln_io")
            nc.sync.dma_start(out=ld, in_=x2c_d[_ts(tb, 128), :])
            nc.vector.tensor_add(ld, ld, b2)
            for nh2 in range(2):
                ps = pp.tile([128, 512], F32, tag="pp")
                for hb in range(16):
                    nc.tensor.matmul(ps, hT[:, hb, _ts(tb, 128)],
                                     w2a[:, hb, _ts(nh2, 512)],
                                     start=(hb == 0), stop=False)
                for hb in range(16):
                    nc.tensor.matmul(ps, hT[:, 16 + hb, _ts(tb, 128)],
                                     w2b[:, hb, _ts(nh2, 512)],
                                     start=False, stop=(hb == 15))
                nc.vector.tensor_add(ld[:, _ts(nh2, 512)], ld[:, _ts(nh2, 512)], ps)
            ln_tile(ld, 3)
            nc.sync.dma_start(out=out.ap()[_ts(tb, 128), :], in_=ld)


# ====================== host side ======================

def make_causal_masks():
    m = np.zeros((4, 128, 512), dtype=np.float32)
    pk = np.arange(128)[:, None]
    pq = np.arange(512)[None, :]
    for i in range(4):
        m[i] = (pk <= pq - 128 * i).astype(np.float32)
    return m.astype(ml_dtypes.bfloat16)


def shard_inputs(inputs, num_devices=8):
    bf = ml_dtypes.bfloat16
    f32 = np.float32
    G = 4
    cmask = make_causal_masks()
    inp = {k: np.asarray(v) for k, v in inputs.items()}
    S = inp["input"].shape[1]
    CH = S // G
    in_maps = []
    xT_c, encT_c = {}, {}
    for c in range(num_devices):
        g, r = c // G, c % G
        if g not in xT_c:
            xT_c[g] = np.ascontiguousarray(inp["input"][g].T.astype(bf))
            encT_c[g] = np.ascontiguousarray(inp["encoder_output"][g].T.astype(bf))
        qs = slice(r * SA_QKV, (r + 1) * SA_QKV)
        m = {
            "x0T_b": xT_c[g],
            "x0c_f": inp["input"][g][r * CH:(r + 1) * CH].astype(f32),
            "encT_b": encT_c[g],
            "sa_wq": inp["sa_wq"][:, qs].astype(bf),
            "sa_wk": inp["sa_wk"][:, qs].astype(bf),
            "sa_wv": inp["sa_wv"][:, qs].astype(bf),
            "sa_wo": inp["sa_wo"][qs, :].astype(bf),
            "sa_bq": inp["sa_bq"][qs].astype(f32),
            "sa_bk": inp["sa_bk"][qs].astype(f32),
            "sa_bv": inp["sa_bv"][qs].astype(f32),
            "sa_bo4": (inp["sa_bo"] / G).astype(f32),
            "ca_wq": inp["ca_wq"].astype(bf),
            "ca_wk": inp["ca_wk"].astype(bf),
            "ca_wv": inp["ca_wv"].astype(bf),
            "ca_wo": inp["ca_wo"].astype(bf),
            "ca_bq": inp["ca_bq"].astype(f32),
            "ca_bk": inp["ca_bk"].astype(f32),
            "ca_bv": inp["ca_bv"].astype(f32),
            "ca_bo": inp["ca_bo"].astype(f32),
            "w1": inp["ffn_w1"].astype(bf),
            "b1": inp["ffn_b1"].astype(f32),
            "w2": inp["ffn_w2"].astype(bf),
            "b2": inp["ffn_b2"].astype(f32),
            "cmask": cmask,
        }
        for i in (1, 2, 3):
            m[f"ln{i}_g"] = inp[f"ln{i}_g"].astype(f32)
            m[f"ln{i}_b"] = inp[f"ln{i}_b"].astype(f32)
        in_maps.append(m)
    return in_maps


_NC_CACHE = {}


def _get_nc(S):
    if S not in _NC_CACHE:
        _NC_CACHE[S] = build_decoder_nc(S)
    return _NC_CACHE[S]


def kernel(**inputs):
    x = np.asarray(inputs["input"])
    B, S, _ = x.shape
    nc = _get_nc(S)
    in_maps = shard_inputs(inputs)
    res = bass_utils.run_bass_kernel_spmd(nc, in_maps, core_ids=list(range(8)))
    outb = [np.concatenate([res.results[g * 4 + r]["out"] for r in range(4)], axis=0)
            for g in range(B)]
    return np.stack(outb, axis=0).astype(np.float32)


# revision 31
# speedup vs baseline: 41.6938x; 6.0164x over previous
"""Transformer decoder layer (causal self-attn + cross-attn + FFN, 3 post-LNs)
on 8 Trainium2 NeuronCores — single-collective design.

Sharding: 2-way data parallel (batch) x 4-way within each batch group.
  core c: batch g = c // 4, rank r = c % 4; chunk = tokens [r*512, (r+1)*512).
  - self-attention: tensor parallel over heads (4 of 16 per core),
    ReduceScatter after the output projection -> each core holds its
    512-token chunk of the attention output (the ONLY collective).
  - LN1/LN2/LN3: chunk-local.
  - cross-attention: sequence parallel — all 16 heads for the own 512-token
    query chunk; encoder K/V computed full-width on every core (overlaps
    the ReduceScatter window). No collective needed.
  - FFN: chunk-local with full weights (w1/w2 streamed/parked in slots
    vacated by earlier phases). No collective needed.
  - output: each core writes its own [512, E] chunk; host concatenates.

Host passes x0 and enc pre-transposed ([E, S] feature-major) so no DMA
transposes are needed; x1/x2 chunk transposes are done on the PE.

Attention: scores [128 k, 512 q] blocks, exp via scalar engine, softmax
along k with a ones-column rowsum folded into the o-eviction scale;
probability tiles are streamed (4-deep pool) into the PV accumulation.
"""

import numpy as np
import ml_dtypes

import concourse.bass as bass
import concourse.bacc as bacc
import concourse.tile as tile
from concourse import mybir
from concourse import bass_utils
from concourse.masks import make_identity

F32 = mybir.dt.float32
BF16 = mybir.dt.bfloat16
AF = mybir.ActivationFunctionType
ALU = mybir.AluOpType

E = 1024
H = 16                  # total heads
SA_HPC = 4              # SA heads per core (16 / 4 ranks)
DK = 64
SA_QKV = SA_HPC * DK    # 256
EB = E // 128           # 8 E partition-blocks
FH = 4096               # FFN hidden


def _ts(i, n):
    return slice(i * n, (i + 1) * n)


def _pbcast(ap, p=128):
    """Broadcast a 1D DRAM AP across p partitions (partition step 0)."""
    return bass.AP(tensor=ap.tensor, offset=ap.offset, ap=[[0, p]] + list(ap.ap))


PHASES = ["null", "saqkv", "sa", "cakv", "rs1", "ln1", "caq", "ca",
          "ln2", "ffn1", "full"]


def build_decoder_nc(S: int, num_devices: int = 8, stop_after: str | None = None,
                     repeat: int = 1):
    assert S % 512 == 0
    nc = bacc.Bacc("TRN2", target_bir_lowering=False, debug=False,
                   num_devices=num_devices)

    G = 4
    CH = S // G            # chunk tokens per core

    din = {}

    def inp(name, shape, dt):
        din[name] = nc.dram_tensor(name, list(shape), dt, kind="ExternalInput")
        return din[name]

    inp("x0T_b", [E, S], BF16)
    inp("x0c_f", [CH, E], F32)
    inp("encT_b", [E, S], BF16)

    inp("sa_wq", [E, SA_QKV], BF16)
    inp("sa_wk", [E, SA_QKV], BF16)
    inp("sa_wv", [E, SA_QKV], BF16)
    inp("sa_wo", [SA_QKV, E], BF16)
    inp("sa_bq", [SA_QKV], F32)
    inp("sa_bk", [SA_QKV], F32)
    inp("sa_bv", [SA_QKV], BF16)
    inp("sa_bo4", [E], F32)          # bo / 4 (summed by the ReduceScatter)

    inp("ca_wq", [E, E], BF16)
    inp("ca_wk", [E, E], BF16)
    inp("ca_wv", [E, E], BF16)
    inp("ca_wo", [E, E], BF16)
    inp("ca_bq", [E], F32)
    inp("ca_bk", [E], F32)
    inp("ca_bv", [E], BF16)
    inp("ca_bo", [E], F32)

    inp("w1", [E, FH], BF16)
    inp("b1", [FH], F32)
    inp("w2", [FH, E], BF16)
    inp("b2", [E], F32)
    for i in (1, 2, 3):
        inp(f"ln{i}_g", [E], F32)
        inp(f"ln{i}_b", [E], F32)
    inp("cmask", [4, 128, 512], BF16)

    out = nc.dram_tensor("out", [CH, E], F32, kind="ExternalOutput")

    rg = [[0, 1, 2, 3], [4, 5, 6, 7]][: max(1, num_devices // 4)]
    if num_devices < 8:
        rg = [list(range(num_devices))]

    with tile.TileContext(nc) as tc:
        for rep in range(repeat):
            _emit(tc, din, out, S, rg, stop_after, sfx=f"_r{rep}")

    nc.compile()
    return nc


def _emit(tc, din, out, S, rg, stop_after=None, sfx=""):
    nc = tc.nc
    TB = S // 128          # k token blocks
    QT = S // 512          # SA query tiles
    CH = S // 4            # chunk tokens
    CB = CH // 128         # chunk token blocks

    def cut(phase):
        return stop_after == phase

    def finish():
        nc.sync.dma_start(out=out.ap(), in_=din["x0c_f"].ap())

    with (
        tc.tile_pool(name="const" + sfx, bufs=1) as const,
        tc.tile_pool(name="wpool" + sfx, bufs=1) as wpool,
        tc.tile_pool(name="bigA" + sfx, bufs=1) as bigA,
        tc.tile_pool(name="bigB" + sfx, bufs=1) as bigB,
        tc.tile_pool(name="bigC" + sfx, bufs=1) as bigC,
        tc.tile_pool(name="qk" + sfx, bufs=1) as qk_pool,
        tc.tile_pool(name="at" + sfx, bufs=3) as at_pool,
        tc.tile_pool(name="denp" + sfx, bufs=2) as denp,
        tc.tile_pool(name="ws" + sfx, bufs=2) as ws,
        tc.tile_pool(name="xtc" + sfx, bufs=1) as xtc_pool,
        tc.tile_pool(name="lnp" + sfx, bufs=2) as lnp,
        tc.tile_pool(name="stat" + sfx, bufs=8) as stat,
        tc.tile_pool(name="pp" + sfx, bufs=2, space="PSUM") as pp,
        tc.tile_pool(name="ps_s" + sfx, bufs=2, space="PSUM") as ps_s,
        tc.tile_pool(name="po" + sfx, bufs=2, space="PSUM") as po_pool,
        tc.tile_pool(name="dram" + sfx, bufs=1, space="DRAM") as dram,
    ):
        # ------------- critical input loads first (DMA-channel priority) ----
        x0T = bigA.tile([128, EB, S], BF16, tag="bigA", name="x0T" + sfx)
        nc.sync.dma_start(out=x0T[:, 0:4, :], in_=din["x0T_b"].ap()[0:512, :]
                          .rearrange("(eb p) s -> p eb s", p=128))
        nc.scalar.dma_start(out=x0T[:, 4:8, :], in_=din["x0T_b"].ap()[512:1024, :]
                            .rearrange("(eb p) s -> p eb s", p=128))
        encT = bigB.tile([128, EB, S], BF16, tag="bigB", name="encT" + sfx)
        nc.sync.dma_start(out=encT[:, 0:4, :], in_=din["encT_b"].ap()[0:512, :]
                          .rearrange("(eb p) s -> p eb s", p=128))
        nc.scalar.dma_start(out=encT[:, 4:8, :], in_=din["encT_b"].ap()[512:1024, :]
                            .rearrange("(eb p) s -> p eb s", p=128))

        sa_w = {}
        for nm in ("wq", "wk", "wv"):
            t = wpool.tile([128, EB, SA_QKV], BF16, tag=nm, name=f"sa_{nm}{sfx}")
            nc.sync.dma_start(out=t, in_=din[f"sa_{nm}"].ap().rearrange(
                "(eb p) m -> p eb m", p=128))
            sa_w[nm] = t
        sa_wo = wpool.tile([128, 2, E], BF16, tag="wo", name=f"sa_wo{sfx}")
        nc.sync.dma_start(out=sa_wo, in_=din["sa_wo"].ap().rearrange(
            "(j p) n -> p j n", p=128))

        # ---------------- constants ----------------
        ident = const.tile([128, 128], BF16)
        make_identity(nc, ident)
        eps_t = const.tile([128, 1], F32)
        nc.vector.memset(eps_t, 1e-12)
        cmask = const.tile([128, 4, 512], BF16)
        nc.sync.dma_start(out=cmask, in_=din["cmask"].ap().rearrange("i p q -> p i q"))

        _bcast_cache = {}

        def bcast(name, tag):
            if name in _bcast_cache:
                return _bcast_cache[name]
            t = const.tile([128, E], F32, name=f"bc_{name}{sfx}", tag=tag)
            nc.sync.dma_start(out=t, in_=_pbcast(din[name].ap()))
            _bcast_cache[name] = t
            return t

        def pp_bias(name, nj):
            t = const.tile([128, nj], F32, name=f"ppb_{name}{sfx}", tag=f"ppb_{name}")
            nc.sync.dma_start(out=t, in_=din[name].ap().rearrange("(j p) -> p j", p=128))
            return t

        sa_bq = pp_bias("sa_bq", 2)
        sa_bk = pp_bias("sa_bk", 2)
        ca_bq = pp_bias("ca_bq", 8)
        ca_bk = pp_bias("ca_bk", 8)
        b1_t = pp_bias("b1", FH // 128)

        ones_r = const.tile([1, 128], BF16, name=f"ones_r{sfx}", tag="ones_r")
        nc.vector.memset(ones_r, 1.0)

        def bvr(name, n):
            t = const.tile([1, n], BF16, name=f"bvr_{name}{sfx}", tag="bvr")
            nc.sync.dma_start(out=t[0:1, :], in_=_pbcast(din[name].ap(), p=1))
            return t

        # ---------------- DRAM scratch ----------------
        ar1_in = dram.tile([S, E], BF16, name="ar1_in" + sfx)
        rs1_out = dram.tile([CH, E], BF16, name="rs1_out" + sfx)
        x1c_d = dram.tile([CH, E], F32, name="x1c" + sfx)
        x2c_d = dram.tile([CH, E], F32, name="x2c" + sfx)

        # ---------------- helpers ----------------
        def proj_qk(xT, w, b, dst, nj, nt):
            # dst [128, nj, nt*512] feature-major = w.T @ xT + b
            for j in range(nj):
                for tt in range(nt):
                    ps = pp.tile([128, 512], F32, tag="pp")
                    for eb in range(EB):
                        nc.tensor.matmul(ps, w[:, eb, _ts(j, 128)],
                                         xT[:, eb, _ts(tt, 512)],
                                         start=(eb == 0), stop=(eb == EB - 1))
                    nc.scalar.activation(dst[:, j, _ts(tt, 512)], ps, AF.Identity,
                                         bias=b[:, j:j + 1])

        _wv_cache = {}

        def proj_v(xT, w_dram, bvr_t, dst, nh, vh=0, tbs=None):
            # emit ONE vw-wide chunk (heads vh*vw/64 ..) of the V projection,
            # for token blocks `tbs` (default all). Bias folded into PSUM via
            # a rank-1 ones (x) bias matmul (evict = Act Copy, no DVE).
            vw = min(512, nh * DK)
            if vh == 0 and tbs is None or (tbs and 0 in tbs and vh == 0):
                nc.vector.memset(dst[:, :, :, 64:65], 1.0)
            key = (id(dst), vh)
            if key in _wv_cache:
                wv = _wv_cache[key]
            else:
                wv = ws.tile([128, EB, vw], BF16, tag="ws")
                nc.sync.dma_start(out=wv, in_=w_dram[:, _ts(vh, vw)].rearrange(
                    "(eb p) m -> p eb m", p=128))
                _wv_cache[key] = wv
            for tb in (range(TB) if tbs is None else tbs):
                ps = pp.tile([128, vw], F32, tag="pp")
                nc.tensor.matmul(ps, ones_r[0:1, :], bvr_t[0:1, _ts(vh, vw)],
                                 start=True, stop=False)
                for eb in range(EB):
                    nc.tensor.matmul(ps, xT[:, eb, _ts(tb, 128)], wv[:, eb, :],
                                     start=False, stop=(eb == EB - 1))
                hs = slice(vh * (vw // DK), (vh + 1) * (vw // DK))
                nc.scalar.activation(
                    dst[:, tb, hs, 0:64],
                    ps.rearrange("p (h d) -> p h d", d=64), AF.Copy)

        def attention(qT, kT, v, oT, heads, nqt, causal):
            # qT [128, nh/2, nqt*512]; kT [128, nh/2, S]; v [128, TB, nh, 65]
            # oT [128, nh/2, nqt*512] written directly (d-major): the PV matmul
            # keeps v stationary so out is [65, 512q]; row 64 is the softmax
            # denominator, divided out per q column via a partition broadcast.
            for h in heads:
                hp = slice((h % 2) * 64, (h % 2) * 64 + 64)
                j = h // 2
                for qt in range(nqt):
                    kb_max = min(TB, 4 * qt + 4) if causal else TB
                    po = po_pool.tile([65, 512], F32, tag="po")
                    for kp in range(kb_max // 2):
                        ps2 = ps_s.tile([128, 2, 512], F32, tag="ps_s")
                        for z in range(2):
                            nc.tensor.matmul(ps2[:, z, :],
                                             kT[hp, j, _ts(2 * kp + z, 128)],
                                             qT[hp, j, _ts(qt, 512)],
                                             start=True, stop=True)
                        at = at_pool.tile([128, 2, 512], BF16, tag="at")
                        nc.scalar.activation(at, ps2, AF.Exp, scale=0.125)
                        for z in range(2):
                            kb = 2 * kp + z
                            if causal and kb >= 4 * qt:
                                nc.vector.tensor_mul(at[:, z, :], at[:, z, :],
                                                     cmask[:, kb - 4 * qt, :])
                            nc.tensor.matmul(po, v[:, kb, h, :], at[:, z, :],
                                             start=(kb == 0),
                                             stop=(kb == kb_max - 1))
                    den = denp.tile([1, 512], F32, tag="den")
                    nc.vector.reciprocal(den, po[64:65, :])
                    denb = denp.tile([64, 512], F32, tag="denb")
                    nc.gpsimd.partition_broadcast(denb, den)
                    nc.vector.tensor_mul(oT[hp, j, _ts(qt, 512)], po[0:64, :], denb)

        def ln_tile(ld, i):
            # in-place layernorm of ld [128, E] with ln{i} params
            st = stat.tile([128, 2, 6], F32, tag="bnst")
            for sg in range(2):
                nc.vector.bn_stats(st[:, sg, :], ld[:, _ts(sg, 512)])
            mv = stat.tile([128, 2], F32, tag="bnmv")
            nc.vector.bn_aggr(mv, st)
            sd = stat.tile([128, 1], F32, tag="sd")
            nc.scalar.activation(sd, mv[:, 1:2], AF.Sqrt, bias=eps_t)
            rstd = stat.tile([128, 1], F32, tag="rstd")
            nc.vector.reciprocal(rstd, sd)
            nc.vector.tensor_scalar(ld, ld, mv[:, 0:1], rstd,
                                    ALU.subtract, ALU.mult)
            nc.vector.tensor_mul(ld, ld, bcast(f"ln{i}_g", "lng"))
            nc.vector.tensor_add(ld, ld, bcast(f"ln{i}_b", "lnb"))

        def transpose_chunk(xb_tb, xTc, tb):
            # xb_tb [128, E] bf16 token-major -> xTc[:, eb, tb*128:...]
            for eb in range(EB):
                pt = po_pool.tile([128, 128], BF16, tag="po")
                nc.tensor.transpose(pt, xb_tb[:, _ts(eb, 128)], ident)
                nc.vector.tensor_copy(xTc[:, eb, _ts(tb, 128)], pt)

        # ================= SA (TP over heads) =================
        if cut("null"):
            finish()
            return

        qT = qk_pool.tile([128, 2, S], BF16, tag="qT", name="sa_qT" + sfx)
        kT = qk_pool.tile([128, 2, S], BF16, tag="kT", name="sa_kT" + sfx)
        v = bigC.tile([128, TB, SA_HPC, 65], BF16, tag="bigC", name="sa_v" + sfx)
        proj_qk(x0T, sa_w["wq"], sa_bq, qT, 2, QT)
        proj_qk(x0T, sa_w["wk"], sa_bk, kT, 2, QT)
        proj_v(x0T, din["sa_wv"].ap(), bvr("sa_bv", SA_QKV), v, SA_HPC)  # vh=0 only

        if cut("saqkv"):
            finish()
            return

        oT = qk_pool.tile([128, 2, S], BF16, tag="oT", name="sa_oT" + sfx)
        attention(qT, kT, v, oT, range(SA_HPC), QT, causal=True)

        # out_proj partials (+ bo/4) -> ar1_in
        bo4 = bcast("sa_bo4", "bo")
        for tb in range(TB):
            y = lnp.tile([128, E], BF16, tag="ln_bf")
            for nh2 in range(2):
                ps = pp.tile([128, 512], F32, tag="pp")
                for jj in range(2):
                    nc.tensor.matmul(ps, oT[:, jj, _ts(tb, 128)],
                                     sa_wo[:, jj, _ts(nh2, 512)],
                                     start=(jj == 0), stop=(jj == 1))
                nc.vector.tensor_add(y[:, _ts(nh2, 512)], ps, bo4[:, _ts(nh2, 512)])
            nc.sync.dma_start(out=ar1_in[_ts(tb, 128), :], in_=y)

        if cut("sa"):
            finish()
            return

        # ========== encoder K/V (full width; overlaps RS1) ==========
        ekT = bigA.tile([128, EB, S], BF16, tag="bigA", name="ekT" + sfx)
        for j in range(EB):
            wk = ws.tile([128, EB, 128], BF16, tag="ws")
            nc.sync.dma_start(out=wk, in_=din["ca_wk"].ap()[:, _ts(j, 128)].rearrange(
                "(eb p) m -> p eb m", p=128))
            for tt in range(QT):
                ps = pp.tile([128, 512], F32, tag="pp")
                for eb in range(EB):
                    nc.tensor.matmul(ps, wk[:, eb, :], encT[:, eb, _ts(tt, 512)],
                                     start=(eb == 0), stop=(eb == EB - 1))
                nc.scalar.activation(ekT[:, j, _ts(tt, 512)], ps, AF.Identity,
                                     bias=ca_bk[:, j:j + 1])
        ev = bigC.tile([128, TB, H, 65], BF16, tag="bigC", name="ev" + sfx)
        bvr_ca = bvr("ca_bv", E)
        proj_v(encT, din["ca_wv"].ap(), bvr_ca, ev, H, vh=0)

        if cut("cakv"):
            finish()
            return

        nc.gpsimd.collective_compute(
            "ReduceScatter", ALU.add, replica_groups=rg,
            ins=[ar1_in.opt()], outs=[rs1_out.opt()])

        if cut("rs1"):
            finish()
            return

        # ========== LN1 (chunk) + x1 transpose ==========
        x1Tc = xtc_pool.tile([128, EB, CH], BF16, tag="xtc", name="x1Tc" + sfx)
        for tb in range(CB):
            ld = lnp.tile([128, E], F32, tag="ln_io")
            arb = lnp.tile([128, E], BF16, tag="ln_bf")
            nc.sync.dma_start(out=arb, in_=rs1_out[_ts(tb, 128), :])
            nc.sync.dma_start(out=ld, in_=din["x0c_f"].ap()[_ts(tb, 128), :])
            nc.vector.tensor_add(ld, ld, arb)
            ln_tile(ld, 1)
            nc.sync.dma_start(out=x1c_d[_ts(tb, 128), :], in_=ld)
            xb = lnp.tile([128, E], BF16, tag="ln_bf")
            nc.vector.tensor_copy(xb, ld)
            transpose_chunk(xb, x1Tc, tb)

        if cut("ln1"):
            finish()
            return

        # ========== CA (sequence parallel, all heads) ==========
        qTc = qk_pool.tile([128, EB, CH], BF16, tag="qT", name="ca_qTc" + sfx)
        for j in range(EB):
            wq = ws.tile([128, EB, 128], BF16, tag="ws")
            nc.sync.dma_start(out=wq, in_=din["ca_wq"].ap()[:, _ts(j, 128)].rearrange(
                "(eb p) m -> p eb m", p=128))
            ps = pp.tile([128, CH], F32, tag="pp")
            for eb in range(EB):
                nc.tensor.matmul(ps, wq[:, eb, :], x1Tc[:, eb, :],
                                 start=(eb == 0), stop=(eb == EB - 1))
            nc.scalar.activation(qTc[:, j, :], ps, AF.Identity,
                                 bias=ca_bq[:, j:j + 1])

        if cut("caq"):
            finish()
            return

        oTc = qk_pool.tile([128, EB, CH], BF16, tag="oT", name="ca_oTc" + sfx)
        attention(qTc, ekT, ev, oTc, range(8), 1, causal=False)
        proj_v(encT, din["ca_wv"].ap(), bvr_ca, ev, H, vh=1)
        wo0 = ws.tile([128, EB, 512], BF16, tag="ws")
        nc.sync.dma_start(out=wo0, in_=din["ca_wo"].ap()[:, 0:512].rearrange(
            "(j p) n -> p j n", p=128))
        wo1 = ws.tile([128, EB, 512], BF16, tag="ws")
        nc.sync.dma_start(out=wo1, in_=din["ca_wo"].ap()[:, 512:1024].rearrange(
            "(j p) n -> p j n", p=128))
        attention(qTc, ekT, ev, oTc, range(8, 16), 1, causal=False)

        if cut("ca"):
            finish()
            return

        # ========== CA out_proj + LN2 (chunk) + x2 transpose ==========
        ca_bo = bcast("ca_bo", "bo")
        x2Tc = xtc_pool.tile([128, EB, CH], BF16, tag="xtc", name="x2Tc" + sfx)
        for tb in range(CB):
            ld = lnp.tile([128, E], F32, tag="ln_io")
            nc.sync.dma_start(out=ld, in_=x1c_d[_ts(tb, 128), :])
            nc.vector.tensor_add(ld, ld, ca_bo)
            for nh2, wo in ((0, wo0), (1, wo1)):
                ps = pp.tile([128, 512], F32, tag="pp")
                for jj in range(EB):
                    nc.tensor.matmul(ps, oTc[:, jj, _ts(tb, 128)], wo[:, jj, :],
                                     start=(jj == 0), stop=(jj == EB - 1))
                nc.vector.tensor_add(ld[:, _ts(nh2, 512)], ld[:, _ts(nh2, 512)], ps)
            ln_tile(ld, 2)
            nc.sync.dma_start(out=x2c_d[_ts(tb, 128), :], in_=ld)
            xb = lnp.tile([128, E], BF16, tag="ln_bf")
            nc.vector.tensor_copy(xb, ld)
            transpose_chunk(xb, x2Tc, tb)

        if cut("ln2"):
            finish()
            return

        # ========== FFN (chunk-local, full weights) ==========
        # w2 parks in the slots vacated by ekT (bigA) and encT (bigB)
        w2a = bigA.tile([128, 16, E], BF16, tag="bigA", name="w2a" + sfx)
        nc.scalar.dma_start(out=w2a, in_=din["w2"].ap()[0:2048, :].rearrange(
            "(hb p) n -> p hb n", p=128))
        w2b = bigB.tile([128, 16, E], BF16, tag="bigB", name="w2b" + sfx)
        nc.scalar.dma_start(out=w2b, in_=din["w2"].ap()[2048:4096, :].rearrange(
            "(hb p) n -> p hb n", p=128))

        hT = bigC.tile([128, FH // 128, CH], BF16, tag="bigC", name="hT" + sfx)
        for hc in range(FH // 512):
            w1c = ws.tile([128, EB, 512], BF16, tag="ws")
            nc.sync.dma_start(out=w1c, in_=din["w1"].ap()[:, _ts(hc, 512)].rearrange(
                "(eb p) m -> p eb m", p=128))
            for hl in range(4):
                hb = hc * 4 + hl
                ps = pp.tile([128, 512], F32, tag="pp")
                for eb in range(EB):
                    nc.tensor.matmul(ps, w1c[:, eb, _ts(hl, 128)], x2Tc[:, eb, :],
                                     start=(eb == 0), stop=(eb == EB - 1))
                nc.scalar.activation(hT[:, hb, :], ps, AF.Relu,
                                     bias=b1_t[:, hb:hb + 1])

        if cut("ffn1"):
            finish()
            return

        b2 = bcast("b2", "bo")
        for tb in range(CB):
            ld = lnp.tile([128, E], F32, tag="ln_io")
            nc.sync.dma_start(out=ld, in_=x2c_d[_ts(tb, 128), :])
            nc.vector.tensor_add(ld, ld, b2)
            for nh2 in range(2):
                ps = pp.tile([128, 512], F32, tag="pp")
                for hb in range(16):
                    nc.tensor.matmul(ps, hT[:, hb, _ts(tb, 128)],
                                     w2a[:, hb, _ts(nh2, 512)],
                                     start=(hb == 0), stop=False)
                for hb in range(16):
                    nc.tensor.matmul(ps, hT[:, 16 + hb, _ts(tb, 128)],
                                     w2b[:, hb, _ts(nh2, 512)],
                                     start=False, stop=(hb == 15))
                nc.vector.tensor_add(ld[:, _ts(nh2, 512)], ld[:, _ts(nh2, 512)], ps)
            ln_tile(ld, 3)
            nc.sync.dma_start(out=out.ap()[_ts(tb, 128), :], in_=ld)


# ====================== host side ======================

def make_causal_masks():
    m = np.zeros((4, 128, 512), dtype=np.float32)
    pk = np.arange(128)[:, None]
    pq = np.arange(512)[None, :]
    for i in range(4):
        m[i] = (pk <= pq - 128 * i).astype(np.float32)
    return m.astype(ml_dtypes.bfloat16)


def shard_inputs(inputs, num_devices=8):
    bf = ml_dtypes.bfloat16
    f32 = np.float32
    G = 4
    cmask = make_causal_masks()
    inp = {k: np.asarray(v) for k, v in inputs.items()}
    S = inp["input"].shape[1]
    CH = S // G
    in_maps = []
    xT_c, encT_c = {}, {}
    for c in range(num_devices):
        g, r = c // G, c % G
        if g not in xT_c:
            xT_c[g] = np.ascontiguousarray(inp["input"][g].T.astype(bf))
            encT_c[g] = np.ascontiguousarray(inp["encoder_output"][g].T.astype(bf))
        qs = slice(r * SA_QKV, (r + 1) * SA_QKV)
        m = {
            "x0T_b": xT_c[g],
            "x0c_f": inp["input"][g][r * CH:(r + 1) * CH].astype(f32),
            "encT_b": encT_c[g],
            "sa_wq": inp["sa_wq"][:, qs].astype(bf),
            "sa_wk": inp["sa_wk"][:, qs].astype(bf),
            "sa_wv": inp["sa_wv"][:, qs].astype(bf),
            "sa_wo": inp["sa_wo"][qs, :].astype(bf),
            "sa_bq": inp["sa_bq"][qs].astype(f32),
            "sa_bk": inp["sa_bk"][qs].astype(f32),
            "sa_bv": inp["sa_bv"][qs].astype(bf),
            "sa_bo4": (inp["sa_bo"] / G).astype(f32),
            "ca_wq": inp["ca_wq"].astype(bf),
            "ca_wk": inp["ca_wk"].astype(bf),
            "ca_wv": inp["ca_wv"].astype(bf),
            "ca_wo": inp["ca_wo"].astype(bf),
            "ca_bq": inp["ca_bq"].astype(f32),
            "ca_bk": inp["ca_bk"].astype(f32),
            "ca_bv": inp["ca_bv"].astype(bf),
            "ca_bo": inp["ca_bo"].astype(f32),
            "w1": inp["ffn_w1"].astype(bf),
            "b1": inp["ffn_b1"].astype(f32),
            "w2": inp["ffn_w2"].astype(bf),
            "b2": inp["ffn_b2"].astype(f32),
            "cmask": cmask,
        }
        for i in (1, 2, 3):
            m[f"ln{i}_g"] = inp[f"ln{i}_g"].astype(f32)
            m[f"ln{i}_b"] = inp[f"ln{i}_b"].astype(f32)
        in_maps.append(m)
    return in_maps


_NC_CACHE = {}


def _get_nc(S):
    if S not in _NC_CACHE:
        _NC_CACHE[S] = build_decoder_nc(S)
    return _NC_CACHE[S]


def kernel(**inputs):
    x = np.asarray(inputs["input"])
    B, S, _ = x.shape
    nc = _get_nc(S)
    in_maps = shard_inputs(inputs)
    res = bass_utils.run_bass_kernel_spmd(nc, in_maps, core_ids=list(range(8)))
    outb = [np.concatenate([res.results[g * 4 + r]["out"] for r in range(4)], axis=0)
            for g in range(B)]
    return np.stack(outb, axis=0).astype(np.float32)
